# revision 5
# baseline (speedup 1.0000x reference)
"""COLoRALinear fused kernel for 8 TRN2 NeuronCores (Bass/Tile).

Computation (per reference):
  base_out   = x @ W^T + b                         [B,S,Do]
  shared_out = (x @ As^T) @ Bs^T * SCALING
  routing    = softmax(mean_s(x) @ task_emb^T)     [B,E]
  t          = x @ Ae^T (per expert)               [B,S,E,R]
  task_out   = sum_e routing[b,e] * t_e @ Be^T * SCALING
  out = base_out + cw*shared_out + (1-cw)*task_out,  cw = sigmoid(collab_w)

Sharding: flatten x to [B*S, Din] = [8192, 2048]; core c owns rows
[c*1024, (c+1)*1024) — all from batch b = c//2.  W and the low-rank
params are replicated.  The routing mean needs all of batch b, so each
core reduces its half and a pairwise AllReduce ([[0,1],[2,3],...])
completes the per-batch logits on-device.

On-core algorithm (all matmuls fp16 with fp32 PSUM accumulation):
  warmup: memset-sourced junk matmuls (no DMA dependency) flip the PE
          HAM clock-gate to 2.4GHz during the framework preamble.
  stage1: u[80, m] = Aall @ x_shard^T where Aall stacks
          [task_emb (8); shared_A (8); expert_A (64)], emitted
          interleaved with the first NHELD base chains (oc0) in
          DMA-arrival order so the x/WT load window does real work.
  logits: rows 0:8 of u, reduced over m, pair-AllReduced -> softmax.
          The collective takes ~45-70us end-to-end (rendezvous barrier
          + ncfw setup), so chunk epilogues that need the routing are
          deferred: the NHELD phase-1 chains simply stay open in PSUM
          (zero extra cost), chunks 3..DEFER-1 stage base-only sums to
          SBUF (one extra matmul + add later), the rest run inline.
  scale:  per-row scales for u via a tiny matmul with Emap (folds
          (1-cw)*SCALING*r_e for expert rows, 1.0 for shared rows since
          sum_e r_e = 1; cw*SCALING is folded into C2's shared rows).
  stage2: out_chunk += u_scaled^T @ C2 as the 17th accumulating matmul
          on top of 16 base-matmul K-chunks; C2 row 80 = base_b with a
          ones-row in u_scaled providing the bias.  Output DMAs in fp16
          (|out|<=8, ~1e-4 rounding), host upcasts to f32.

PSUM plan (8 banks): NHELD=3 held chains + 1 u/scale ring + 2 chunk
rotation + 2 drain rotation (the u m-half-1 pass borrows a drain bank).
"""

import numpy as np

import concourse.bass as bass
import concourse.mybir as mybir
import concourse.tile as tile
from concourse import bacc
from concourse.bass import ts
from concourse.bass_utils import run_bass_kernel_spmd

# Problem shapes (hardcoded per spec)
B, S, DIN, DOUT = 4, 2048, 2048, 2048
E, R = 8, 8
SCALING = 16.0 / 8.0
N_CORES = 8
M_CORE = B * S // N_CORES          # 1024 rows per core
P = 128                            # partitions
KT = DIN // P                      # 16 contraction chunks
NOC = DOUT // 512                  # 4 output chunks of 512
NMT = M_CORE // P                  # 8 m-tiles of 128
AW = 80                            # rows of A-stack: 8 taskemb + 8 shared + 64 expert
CW = 81                            # rows of C2: 8 zero + 8 shared + 64 expert + 1 bias
DEFER = 20                         # chunks 3..DEFER-1 staged to SBUF (routing wait)
WQ = 4                             # WT slab split: KT/WQ i-chunks per DMA
NHELD = 3                          # base chains held open in PSUM across the wait
WARMUP_MM = 55                     # junk matmuls to flip the PE HAM clock-gate early

# compute dtype: fp16 runs at the same TensorE rate as bf16 but with 10
# mantissa bits; all operand magnitudes here are in [1e-2, 5] so no
# overflow/subnormal risk (the tiny task_emb/S scale is folded into the
# softmax exp instead of the weights)
BF16 = np.float16

# set by test.py for profiling
TRACE = False
LAST_RESULT = None

_cached = None


def _build_nc():
    nc = bacc.Bacc(
        "TRN2",
        target_bir_lowering=False,
        debug=False,
        num_devices=N_CORES,
    )
    BF = mybir.dt.float16
    F32 = mybir.dt.float32

    # host-packed layouts: partition-major so every DMA reads large
    # contiguous runs per partition
    xT_d = nc.dram_tensor("xT", [DIN, M_CORE], BF, kind="ExternalInput")
    WT_d = nc.dram_tensor("WT", [P, NOC, KT, 512], BF, kind="ExternalInput")
    AallT_d = nc.dram_tensor("AallT", [P, KT, AW], BF, kind="ExternalInput")
    C2_d = nc.dram_tensor("C2", [CW, DOUT], BF, kind="ExternalInput")
    Emap_d = nc.dram_tensor("Emap", [E, AW], BF, kind="ExternalInput")
    # fp16 output: |out| <= ~8 so fp16 rounding (~1e-4 rel) is negligible,
    # and it halves the output DMA bytes + flush tail; host upcasts
    out_d = nc.dram_tensor("out", [M_CORE, DOUT], BF, kind="ExternalOutput")

    ones_d = nc.dram_tensor("ones", [M_CORE], BF, kind="ExternalInput")

    cc_in = nc.dram_tensor("cc_in", [E], F32)
    cc_out = nc.dram_tensor("cc_out", [E], F32)
    r_bounce = nc.dram_tensor("r_bounce", [E], BF)

    X = mybir.AxisListType.X

    with tile.TileContext(nc) as tc:
        with (
            tc.tile_pool(name="consts", bufs=1) as consts,
            tc.tile_pool(name="small", bufs=1) as small,
            # PSUM: 3 held + 1 small (u/scale ring) + 2 chunk rotation +
            # 2 drain rotation = 8 banks.  Separate chunk/drain pools give
            # the chunk ring a full 2-chunk (7us) WAR slack.
            tc.tile_pool(name="pheld", bufs=1, space="PSUM") as pheld,
            tc.tile_pool(name="pmm", bufs=2, space="PSUM") as pmm,
            tc.tile_pool(name="pdrain", bufs=2, space="PSUM") as pdrain,
            tc.tile_pool(name="psmall", bufs=1, space="PSUM") as psmall,
            tc.tile_pool(name="outp", bufs=3) as outp,
        ):
            # ---- PE warmup source: memset, no DMA dependency ----
            # junk matmuls can start right after the framework preamble
            # (~7us), flipping the HAM clock-gate to 2.4GHz before stage-1
            warm_sb = consts.tile([P, P], BF)
            nc.vector.memset(warm_sb[:, :], 0.25)

            # ---- constant / input loads ----
            # One FIFO HW queue services all sync-engine DMAs (issue order
            # == start order, ~0.6us issue each, 8 shared completion
            # lanes), so keep the proven baseline order: x slabs stream
            # consecutively for tight stage-1 pacing, WT oc0 quarters slot
            # between the halves, everything else follows.
            # AallT rides the Scalar HWDGE queue: a parallel hardware
            # queue, idle this early, so AallT lands at wire speed
            # (~1us) without spending Sync-wire bytes.  (On the gpsimd
            # SWDGE it moved at ~65GB/s and landed at 13.6us, 2.5us
            # after the PE went idle — which also tripped the HAM
            # clock down to half rate.)
            AallT_sb = consts.tile([P, KT, AW], BF)
            nc.scalar.dma_start(AallT_sb[:, :, :], AallT_d[:, :, :])
            xT_sb = consts.tile([P, KT, M_CORE], BF)
            WT_sb = consts.tile([P, NOC, KT, 512], BF)

            def x_load(i):
                nc.sync.dma_start(xT_sb[:, i, :], xT_d[ts(i, P), :])

            def wt_load(oc, iq, nq=1):
                nc.sync.dma_start(
                    WT_sb[:, oc, iq * WQ : (iq + nq) * WQ, :],
                    WT_d[:, oc, iq * WQ : (iq + nq) * WQ, :],
                )

            for i in range(0, 8):
                x_load(i)
            wt_load(0, 0)
            wt_load(0, 1)
            for i in range(8, KT):
                x_load(i)
            wt_load(0, 2)
            wt_load(0, 3)
            wt_load(1, 0, nq=2)
            wt_load(1, 2, nq=2)
            Emap_sb = consts.tile([E, AW], BF)
            nc.sync.dma_start(Emap_sb[:], Emap_d[:, :])
            C2_sb = consts.tile([CW, DOUT], BF)
            nc.sync.dma_start(C2_sb[:], C2_d[:, :])
            for oc in range(2, NOC):
                wt_load(oc, 0, nq=2)
                wt_load(oc, 2, nq=2)

            # ---- PE warmup ----
            # Results are never read.
            warm_ps = pmm.tile([P, 512], mybir.dt.float32, tag="ps")

            def junk_mm(w):
                nc.tensor.matmul(
                    warm_ps[0:P, 0:AW],
                    warm_sb[:, :],
                    warm_sb[:, 0:AW],
                    start=True,
                    stop=True,
                )

            for w in range(WARMUP_MM):
                junk_mm(w)

            # row-80 bias ones-row: gpsimd queue, issued at t~0 (engine ops
            # need 32-aligned partition bases, DMA does not)
            u_scaled = small.tile([CW, M_CORE], BF)
            nc.gpsimd.dma_start(u_scaled[AW : AW + 1, :], ones_d[:])

            # ---- phase 1: stage1 m-half-0 + the first NHELD base chains
            # (oc0, mt0..), emitted in DMA-arrival order so the PE converts
            # the x/WT load window into real work.  The held chains stay
            # OPEN in PSUM until the routing collective lands (~65us) —
            # their low-rank term is a 17th accumulating matmul, no staging
            # cost.  A-stack rows: 0..7 taskemb, 8..15 shared, 16..79 expert
            u_sb = small.tile([AW, M_CORE], F32)
            held_ps = [
                pheld.tile([P, 512], mybir.dt.float32, tag=f"held{mt}",
                           name=f"held_ps{mt}")
                for mt in range(NHELD)
            ]

            def base_mm(mt, oc, i, ps, stop=False):
                nc.tensor.matmul(
                    ps[:],
                    xT_sb[:, i, ts(mt, P)],
                    WT_sb[:, oc, i, :],
                    start=(i == 0),
                    stop=stop,
                )

            lg_parts = []

            def stage1_half(h, u_ps_h, i):
                nc.tensor.matmul(
                    u_ps_h[:, :],
                    AallT_sb[:, i, :],
                    xT_sb[:, i, ts(h, 512)],
                    start=(i == 0),
                    stop=(i == KT - 1),
                )

            def stage1_evac(h, u_ps_h):
                lg_h = small.tile([E, 1], F32, tag=f"lg{h}")
                nc.vector.reduce_sum(lg_h[0:8, :], u_ps_h[0:8, :], axis=X)
                nc.vector.tensor_copy(u_sb[:, ts(h, 512)], u_ps_h[0:AW, :])
                lg_parts.append(lg_h)

            # emission follows DMA-arrival order: s1 over x0-7 (slab-paced,
            # junk-filled), base quads when the WT oc0 quarters land, s1
            # over x8-15, the rest of the base quads, then m-half-1
            u_ps0 = psmall.tile([AW, 512], mybir.dt.float32, tag="u_ps")
            for i in range(8):
                stage1_half(0, u_ps0, i)
                junk_mm(0)
                junk_mm(1)
            for w in range(4):
                junk_mm(w)
            for ii in range(0, 8):
                for mt in range(NHELD):
                    base_mm(mt, 0, ii, held_ps[mt])
            for i in range(8, KT):
                stage1_half(0, u_ps0, i)
                junk_mm(0)
            stage1_evac(0, u_ps0)
            junk_mm(0)
            junk_mm(1)
            # m-half-1 runs BEFORE the i8-15 base quads: it needs only x
            # (landed), while the quads wait for WT oc0's tail quarters —
            # matches wire-arrival order deterministically.  It borrows a
            # drain-pool bank (drains run ~60us later); reusing u_ps0's
            # bank here lets the scheduler reorder the first write ahead
            # of half-0's readers -> deadlock
            u_ps1 = pdrain.tile([AW, 512], mybir.dt.float32, tag="ps2")
            for i in range(KT):
                stage1_half(1, u_ps1, i)
            stage1_evac(1, u_ps1)
            for ii in range(8, KT):
                for mt in range(NHELD):
                    base_mm(mt, 0, ii, held_ps[mt])
            lg = small.tile([E, 1], F32, tag="lg")
            nc.vector.tensor_add(
                lg[0:8, :], lg_parts[0][0:8, :], lg_parts[1][0:8, :]
            )

            # ---- cross-core logits reduction (pairs share a batch) ----
            # control-path DMAs use gpsimd SWDGE: off the bulk HW queue,
            # so they don't wait behind the WT/x loads
            nc.gpsimd.dma_start(cc_in[:], lg[0:8, 0:1])
            nc.gpsimd.collective_compute(
                "AllReduce",
                mybir.AluOpType.add,
                replica_groups=[[0, 1], [2, 3], [4, 5], [6, 7]],
                ins=[cc_in.ap().opt()],
                outs=[cc_out.ap().opt()],
            )

            # ---- softmax over E on one partition ----
            lrow = small.tile([1, E], F32)
            nc.gpsimd.dma_start(lrow[:], cc_out[:])
            mx = small.tile([1, 1], F32)
            nc.vector.reduce_max(mx[:], lrow[:], axis=X)
            shf = small.tile([1, E], F32)
            nc.vector.tensor_scalar_sub(shf[:], lrow[:], mx[0:1, 0:1])
            ex = small.tile([1, E], F32)
            # logits carry a factor S (mean not yet applied); softmax is
            # shift-invariant so scaling (l - max) by 1/S inside the exp
            # yields exactly softmax(mean-logits)
            nc.scalar.activation(
                ex[:], shf[:], mybir.ActivationFunctionType.Exp, scale=1.0 / S
            )
            sm = small.tile([1, 1], F32)
            nc.vector.reduce_sum(sm[:], ex[:], axis=X)
            ri = small.tile([1, 1], F32)
            nc.vector.reciprocal(ri[:], sm[:])
            rrow = small.tile([1, E], BF)
            nc.vector.tensor_scalar_mul(rrow[:], ex[:], ri[0:1, 0:1])
            nc.gpsimd.dma_start(r_bounce[:], rrow[:])
            rcol = small.tile([E, 1], BF)
            nc.gpsimd.dma_start(rcol[:], r_bounce[:])

            def emit_scale_chain():
                scale_ps = psmall.tile([AW, 1], mybir.dt.float32, tag="u_ps")
                nc.tensor.matmul(
                    scale_ps[:], Emap_sb[:, :], rcol[:], start=True, stop=True
                )
                # on ScalarE (ACT), not DVE: this chain lands mid-stream right
                # when DVE is busiest with evacuation copies + deferred adds;
                # keeping it off DVE avoids a psum-slot WAR stall on the PE
                scale_sb = small.tile([AW, 1], F32)
                nc.scalar.copy(scale_sb[:], scale_ps[:])
                nc.scalar.activation(
                    u_scaled[0:AW, :],
                    u_sb[0:AW, :],
                    mybir.ActivationFunctionType.Copy,
                    scale=scale_sb[0:AW, 0:1],
                )

            def finish_chunk(mt, oc, ps, split=False):
                # 17th accumulating matmul: shared+task low-rank + bias
                nc.tensor.matmul(
                    ps[:],
                    u_scaled[0:CW, ts(mt, P)],
                    C2_sb[0:CW, ts(oc, 512)],
                    start=False,
                    stop=True,
                )
                if split:
                    # final chunk: halve the evac+DMA and spread the two
                    # halves over DVE+Sync / ScalarE+Scalar-HWDGE so both
                    # are in flight concurrently (shorter flush tail)
                    for hh, (cp, dq) in enumerate(
                        [(nc.vector.tensor_copy, nc.sync),
                         (nc.scalar.copy, nc.scalar)]
                    ):
                        ob = outp.tile([P, 256], BF, tag="obs",
                                       name=f"obs{hh}")
                        cp(ob[:], ps[:, ts(hh, 256)])
                        dq.dma_start(
                            out_d[ts(mt, P), oc * 512 + hh * 256:
                                  oc * 512 + (hh + 1) * 256],
                            ob[:],
                        )
                else:
                    ob = outp.tile([P, 512], BF, tag="ob")
                    nc.vector.tensor_copy(ob[:], ps[:])
                    nc.sync.dma_start(out_d[ts(mt, P), ts(oc, 512)], ob[:])

            def finish_deferred(mt, oc, stage_sb):
                # low-rank product into a fresh psum (own pool, so drain
                # MMs never collide with the chunk ring), added to the
                # staged base result on the way out
                ps2 = pdrain.tile([P, 512], mybir.dt.float32, tag="ps2")
                nc.tensor.matmul(
                    ps2[:],
                    u_scaled[0:CW, ts(mt, P)],
                    C2_sb[0:CW, ts(oc, 512)],
                    start=True,
                    stop=True,
                )
                ob = outp.tile([P, 512], BF, tag="ob")
                nc.vector.tensor_add(ob[:], stage_sb[:], ps2[:])
                nc.sync.dma_start(out_d[ts(mt, P), ts(oc, 512)], ob[:])

            # ---- main loop, phase 2 ----
            # Chunks 0..NHELD-1 (oc0) are already accumulated in held PSUM.
            # Chunks NHELD..DEFER-1 finish base-only and stage to SBUF;
            # their low-rank term is added once the routing collective
            # delivers u_scaled.  From DEFER on, the 17th matmul runs
            # inline.
            chunk_list = [(0, mt) for mt in range(NHELD, NMT)] + [
                (oc, mt) for oc in range(1, NOC) for mt in range(NMT)
            ]
            chunk_idx = NHELD
            deferred = []
            with tc.tile_pool(name="defer", bufs=DEFER) as defer_pool:
                for oc, mt in chunk_list:
                    ps = pmm.tile([P, 512], mybir.dt.float32, tag="ps")
                    for i in range(KT):
                        base_mm(mt, oc, i, ps,
                                stop=(chunk_idx < DEFER and i == KT - 1))
                    if chunk_idx < DEFER:
                        stage_sb = defer_pool.tile([P, 512], F32, tag="stage")
                        nc.vector.tensor_copy(stage_sb[:], ps[:])
                        deferred.append((mt, oc, stage_sb))
                    else:
                        finish_chunk(mt, oc, ps,
                                     split=(oc, mt) == chunk_list[-1])
                        # drain deferred chunks gradually so their DVE
                        # adds interleave with ongoing base matmuls
                        for _ in range(2):
                            if deferred:
                                dmt, doc, dsb = deferred.pop(0)
                                finish_deferred(dmt, doc, dsb)
                    chunk_idx += 1
                    if chunk_idx == DEFER:
                        emit_scale_chain()
                        # close the held chains: 17th matmul straight
                        # into the still-open PSUM banks, no staging cost
                        for hmt in range(NHELD):
                            finish_chunk(hmt, 0, held_ps[hmt])
                for dmt, doc, dsb in deferred:
                    finish_deferred(dmt, doc, dsb)

    nc.compile()
    return nc


def _prep_inputs(x, base_W, base_b, shared_A, shared_B, expert_A, expert_B,
                 task_emb, collab_w):
    f = np.float32
    x = np.asarray(x, dtype=f).reshape(B * S, DIN)
    base_W = np.asarray(base_W, dtype=f)
    base_b = np.asarray(base_b, dtype=f)
    shared_A = np.asarray(shared_A, dtype=f)
    shared_B = np.asarray(shared_B, dtype=f)
    expert_A = np.asarray(expert_A, dtype=f)
    expert_B = np.asarray(expert_B, dtype=f)
    task_emb = np.asarray(task_emb, dtype=f)
    cw = float(1.0 / (1.0 + np.exp(-np.asarray(collab_w, dtype=np.float64))))

    # partition-major packed layouts (large contiguous DMA bursts);
    # cast to fp16 BEFORE the transposed copies to halve host memcpy bytes
    # WT[p, oc, i, j] = base_W.T[i*128+p, oc*512+j]
    WT = np.ascontiguousarray(
        base_W.astype(BF16).T.reshape(KT, P, NOC, 512).transpose(1, 2, 0, 3)
    )                                                                # [P,NOC,KT,512]
    # A-stack rows: 0..7 taskemb (logits; the 1/S mean-scale is applied at
    # the softmax exp to keep fp16 operands out of subnormal range),
    # 8..15 shared, 16..79 expert
    A_all = np.concatenate(
        [task_emb, shared_A, expert_A.reshape(E * R, DIN)], axis=0
    )                                                                # [80, DIN]
    # AallT[p, i, a] = A_all[a, i*128+p]
    AallT = np.ascontiguousarray(
        A_all.T.reshape(KT, P, AW).transpose(1, 0, 2)
    ).astype(BF16)                                                   # [P,KT,AW]

    # C2 rows align with u_scaled rows; row 80 = bias via ones-row
    C2 = np.zeros((CW, DOUT), dtype=f)
    C2[8:16] = shared_B.T * (cw * SCALING)
    C2[16:80] = expert_B.transpose(0, 2, 1).reshape(E * R, DOUT)
    C2[80] = base_b
    C2 = C2.astype(BF16)

    # scale[j] = sum_e Emap[e, j] * r[e]:
    #   taskemb rows -> 0, shared rows -> 1 (softmax sums to 1),
    #   expert row (e,r) -> (1-cw)*SCALING*r_e
    Emap = np.zeros((E, AW), dtype=f)
    Emap[:, 8:16] = 1.0
    for e in range(E):
        Emap[e, 16 + 8 * e : 24 + 8 * e] = (1.0 - cw) * SCALING
    Emap = Emap.astype(BF16)

    ones = np.ones((M_CORE,), dtype=BF16)

    x16 = x.astype(BF16)
    in_maps = []
    for c in range(N_CORES):
        xT = np.ascontiguousarray(x16[c * M_CORE : (c + 1) * M_CORE].T)
        in_maps.append(
            {"xT": xT, "WT": WT, "AallT": AallT, "C2": C2, "Emap": Emap,
             "ones": ones}
        )
    return in_maps


def kernel(**inputs):
    global _cached, LAST_RESULT
    if _cached is None:
        _cached = _build_nc()
    nc = _cached
    in_maps = _prep_inputs(**inputs)
    res = run_bass_kernel_spmd(
        nc, in_maps, core_ids=list(range(N_CORES)), trace=TRACE
    )
    LAST_RESULT = res
    out = np.concatenate(
        [res.results[c]["out"] for c in range(N_CORES)], axis=0
    ).reshape(B, S, DOUT)
    return np.ascontiguousarray(out.astype(np.float32))

